# revision 25
# baseline (speedup 1.0000x reference)
"""Distributed GQA attention prefill kernel for 8 TRN2 NeuronCores.

Problem: llama-style attention, BSZ=2, SEQ=2048, DIM=4096, 32 Q heads,
8 KV heads, head_dim=128, causal prefill (start_pos=0, caches zero).

Sharding: data-parallel over batch (2) x tensor-parallel over heads (4).
Core c = (b, g) with b = c // 4, g = c % 4 handles batch b, Q heads
8g..8g+7, KV heads 2g..2g+1, and wo rows 1024g..1024(g+1). Each core
emits a partial [2048, 4096] output; the host sums the 4 TP partials
per batch. No collectives.

On-chip layout trick: everything is computed in "transposed" layouts so
no activation transpose is ever needed:
  QT[d, t] = wq.T @ x.T       (lhsT = wq natural, rhs = xT from host)
  KT[d, t] = wk.T @ x.T
  V[t, d]  = x @ wv           (lhsT = xT chunk, rhs = wv natural)
  scoresT[kv, q] = K @ QT     (lhsT = KT tile, rhs = QT tile)
  attn[q, d+1]   = P @ [V|1]  (lhsT = expT tile, rhs = V with ones col
                               -> last column accumulates the softmax
                               denominator for free)
RoPE is applied in rotate-half form: the head_dim of wq/wk is permuted
on the host (even dims first, odd dims second) which leaves all dot
products unchanged; cos/sin arrive transposed [64, t].

Schedule: stage A does Q/K/V projection in one x-pass (QT spilled to
DRAM); stage B (attention) then runs with stage C (output projection)
interleaved at head granularity — C's matmuls fill the PE while ScalarE
computes the next head's exp row.
"""

import sys

for p in ("/opt/pypackages", "/opt/trn_rl_repo"):
    if p not in sys.path:
        sys.path.insert(0, p)

import numpy as np
import ml_dtypes

BSZ, SEQ, DIM = 2, 2048, 4096
N_HEADS, N_KV, HD = 32, 8, 128
H_L, KV_L = 8, 2          # per-core local Q heads / KV heads
HL = H_L * HD             # 1024 local head dims
N_CORES = 8
WIN = 256                 # stage-A token window
NW = SEQ // WIN
NK = DIM // 128
NQT = SEQ // 512          # attention q-tiles
NEG = -1e9

_cache = {}


def _build():
    import concourse.mybir as mybir
    import concourse.tile as tile
    from concourse import bacc
    from concourse.masks import make_identity
    from contextlib import ExitStack

    f32 = mybir.dt.float32
    bf16 = mybir.dt.bfloat16
    Exp = mybir.ActivationFunctionType.Exp

    nc = bacc.Bacc()
    xT = nc.declare_dram_parameter("xT", [DIM, SEQ], bf16, isOutput=False)
    wq = nc.declare_dram_parameter("wq", [DIM, HL], bf16, isOutput=False)
    wk = nc.declare_dram_parameter("wk", [DIM, KV_L * HD], bf16, isOutput=False)
    wv = nc.declare_dram_parameter("wv", [DIM, KV_L * HD], bf16, isOutput=False)
    wo = nc.declare_dram_parameter("wo", [HL, DIM], bf16, isOutput=False)
    cosT = nc.declare_dram_parameter("cosT", [64, SEQ], f32, isOutput=False)
    sinT = nc.declare_dram_parameter("sinT", [64, SEQ], f32, isOutput=False)
    maskb = nc.declare_dram_parameter("maskb", [128, 4, 512], f32, isOutput=False)
    out = nc.declare_dram_parameter("out", [SEQ, DIM], f32, isOutput=True)

    qt_dram = nc.dram_tensor("qt_spill", [H_L, HD, SEQ], bf16)

    def dma_split(dst, src, n):
        """Issue n parallel DMAs over the ko axis (dim 1 of dst)."""
        ko = dst.shape[1]
        step = ko // n
        for i in range(n):
            nc.sync.dma_start(
                out=dst[:, i * step:(i + 1) * step],
                in_=src[:, i * step:(i + 1) * step])

    with tile.TileContext(nc) as tc, ExitStack() as res:
        ps_big = res.enter_context(tc.tile_pool(name="ps_big", bufs=4, space="PSUM"))
        ps_att = res.enter_context(tc.tile_pool(name="ps_att", bufs=4, space="PSUM"))
        resid = res.enter_context(tc.tile_pool(name="resid", bufs=1))
        qt_pool = res.enter_context(tc.tile_pool(name="qt", bufs=2))
        qts = {}

        # per-window K/V tiles (window = 512 tokens for attention indexing)
        kt_w = [resid.tile([128, KV_L, 512], bf16, tag=f"kt{w}", name=f"kt{w}")
                for w in range(NQT)]
        v_w = [resid.tile([128, 4, KV_L, 130], bf16, tag=f"v{w}",
                          name=f"v{w}") for w in range(NQT)]
        ident = resid.tile([128, 128], bf16, tag="ident")
        cos_sb = resid.tile([64, SEQ], f32, tag="cos")
        sin_sb = resid.tile([64, SEQ], f32, tag="sin")
        mask_sb = resid.tile([128, 4, 512], f32, tag="mask")

        def rope(ps, dst, t0, tw):
            """dst[0:64]=e*c-o*s ; dst[64:128]=e*s+o*c (e=ps[0:64], o=ps[64:128])."""
            c = cos_sb[:, t0:t0 + tw]
            s = sin_sb[:, t0:t0 + tw]
            t1 = rope_pool.tile([64, WIN], f32, tag="r1", name="r1")[:, :tw]
            t2 = rope_pool.tile([64, WIN], f32, tag="r2", name="r2")[:, :tw]
            nc.vector.tensor_mul(t1, ps[0:64, :tw], c)
            nc.vector.tensor_mul(t2, ps[64:128, :tw], s)
            nc.vector.tensor_sub(dst[0:64, :tw], t1, t2)
            t3 = rope_pool.tile([64, WIN], f32, tag="r1", name="r3")[:, :tw]
            t4 = rope_pool.tile([64, WIN], f32, tag="r2", name="r4")[:, :tw]
            nc.vector.tensor_mul(t3, ps[0:64, :tw], s)
            nc.vector.tensor_mul(t4, ps[64:128, :tw], c)
            nc.vector.tensor_add(dst[64:128, :tw], t3, t4)

        # ---- stage A: Q/K/V projection + RoPE in one x-pass ---------------
        with ExitStack() as sa:
            wq_sb = sa.enter_context(tc.tile_pool(name="wq", bufs=1)).tile(
                [128, NK, HL], bf16, tag="wq")
            wk_sb = sa.enter_context(tc.tile_pool(name="wk", bufs=1)).tile(
                [128, NK, KV_L * HD], bf16, tag="wk")
            wv_sb = sa.enter_context(tc.tile_pool(name="wv", bufs=1)).tile(
                [128, NK, KV_L * HD], bf16, tag="wv")
            xt_pool = sa.enter_context(tc.tile_pool(name="xt", bufs=2))
            rope_pool = sa.enter_context(tc.tile_pool(name="rope", bufs=2))
            qsp_pool = sa.enter_context(tc.tile_pool(name="qsp", bufs=2))

            # critical-path loads first: window-0 xT + wk get the DMA queues
            # to themselves so the first matmul starts in ~2us; wq's 8MB is
            # emitted after window 0's K/V work (Q is last in the window).
            # tiny first chunks so the very first matmul's operands land fast
            xt0 = xt_pool.tile([128, NK, WIN], bf16, tag="xt", name="xt0")
            xt0_src = xT[:, 0:WIN].rearrange("(ko p) t -> p ko t", p=128)
            wk_src = wk.rearrange("(ko p) d -> p ko d", p=128)
            nc.sync.dma_start(out=xt0[:, 0:2], in_=xt0_src[:, 0:2])
            nc.sync.dma_start(out=wk_sb[:, 0:2], in_=wk_src[:, 0:2])
            dma_split(xt0[:, 2:], xt0_src[:, 2:], 6)
            dma_split(wk_sb[:, 2:], wk_src[:, 2:], 6)
            dma_split(wv_sb, wv.rearrange("(ko p) d -> p ko d", p=128), 8)
            nc.sync.dma_start(out=cos_sb, in_=cosT[:, :])
            nc.sync.dma_start(out=sin_sb, in_=sinT[:, :])
            make_identity(nc, ident)
            for w_ in range(NQT):
                nc.vector.memset(v_w[w_][:, :, :, 128:129], 1.0)

            for w in range(NW):
                t0 = w * WIN
                wa, wo512 = t0 // 512, (t0 % 512)
                if w == 0:
                    xt = xt0
                else:
                    xt = xt_pool.tile([128, NK, WIN], bf16, tag="xt", name="xt")
                    dma_split(xt, xT[:, t0:t0 + WIN].rearrange(
                        "(ko p) t -> p ko t", p=128), 4)
                for kh in range(KV_L):
                    ps = ps_big.tile([128, 512], f32, tag="big", name="psk")
                    for k in range(NK):
                        nc.tensor.matmul(
                            ps[:, :WIN], wk_sb[:, k, kh * HD:(kh + 1) * HD],
                            xt[:, k], start=(k == 0), stop=(k == NK - 1))
                    rope(ps, kt_w[wa][:, kh, wo512:wo512 + WIN], t0, WIN)
                for tc_ in range(WIN // 128):
                    ps = ps_big.tile([128, 512], f32, tag="big", name="psv")
                    for k in range(NK):
                        nc.tensor.matmul(
                            ps[:, :KV_L * HD], xt[:, k, tc_ * 128:(tc_ + 1) * 128],
                            wv_sb[:, k], start=(k == 0), stop=(k == NK - 1))
                    for kh in range(KV_L):
                        nc.scalar.copy(
                            v_w[wa][:, wo512 // 128 + tc_, kh, 0:128],
                            ps[:, kh * HD:(kh + 1) * HD])
                if w == 0:
                    dma_split(wq_sb, wq.rearrange("(ko p) d -> p ko d", p=128), 8)
                    nc.sync.dma_start(out=mask_sb, in_=maskb[:, :, :])
                for h in range(H_L):
                    ps = ps_big.tile([128, 512], f32, tag="big", name="psq")
                    for k in range(NK):
                        nc.tensor.matmul(
                            ps[:, :WIN], wq_sb[:, k, h * HD:(h + 1) * HD],
                            xt[:, k], start=(k == 0), stop=(k == NK - 1))
                    qs = qsp_pool.tile([128, WIN], bf16, tag="qs", name="qs")
                    rope(ps, qs, t0, WIN)
                    nc.sync.dma_start(out=qt_dram[h, :, t0:t0 + WIN], in_=qs)
                if t0 + WIN == 1024:
                    # first attention q-tile (q1) fully spilled -> prefetch it
                    qts[1] = qt_pool.tile([128, H_L, 512], bf16, tag="qt",
                                          name="qt0")
                    dma_split(qts[1], qt_dram[:, :, 512:1024]
                              .rearrange("h p q -> p h q"), 2)

        # ---- stage B (attention) with stage C (out-proj) interleaved ------
        with ExitStack() as bc:
            exp_pool = bc.enter_context(tc.tile_pool(name="exp", bufs=8))
            asb_pool = bc.enter_context(tc.tile_pool(name="asb", bufs=8))
            rec_pool = bc.enter_context(tc.tile_pool(name="rec", bufs=8))
            at_sb = bc.enter_context(tc.tile_pool(name="at", bufs=1)).tile(
                [128, H_L, SEQ], bf16, tag="at")
            wo_pool = bc.enter_context(tc.tile_pool(name="wo", bufs=2))
            out_pool = bc.enter_context(tc.tile_pool(name="outp", bufs=4))

            pending = []

            def flush_pending():
                while pending:
                    pending.pop(0)()

            wo_cur = [None]

            def make_strip(qs_):
                """Emission closures for out-proj of token strip qs_ (4 ti)."""
                cls = []
                for di in range(DIM // 512):
                    def load_wo(di=di):
                        wot = wo_pool.tile([128, H_L, 512], bf16, tag="wo",
                                           name="wot")
                        dma_split(wot, wo[:, di * 512:(di + 1) * 512].rearrange(
                            "(ho p) d -> p ho d", p=128), 2)
                        wo_cur[0] = wot
                    cls.append(load_wo)
                    for tj in range(4):
                        def pair(di=di, ti=qs_ * 4 + tj):
                            wot = wo_cur[0]
                            ps = ps_big.tile([128, 512], f32, tag="big",
                                             name="pso")
                            for ho in range(H_L):
                                nc.tensor.matmul(
                                    ps, at_sb[:, ho, ti * 128:(ti + 1) * 128],
                                    wot[:, ho], start=(ho == 0),
                                    stop=(ho == H_L - 1))
                            osb = out_pool.tile([128, 512], f32, tag="osb",
                                                name="osb")
                            nc.vector.tensor_copy(osb, ps)
                            nc.sync.dma_start(
                                out=out[ti * 128:(ti + 1) * 128,
                                        di * 512:(di + 1) * 512],
                                in_=osb)
                        cls.append(pair)
                return cls

            cqueue = []

            # q1 first so its finished strip feeds PE fillers during B(q0)
            order = [1, 0, 2, 3]

            for idx, qi in enumerate(order):
                q0 = qi * 512
                qt = qts.pop(qi)
                if idx + 1 < len(order):
                    nq = order[idx + 1]
                    qts[nq] = qt_pool.tile([128, H_L, 512], bf16,
                                           tag="qt", name="qtn")
                    dma_split(qts[nq], qt_dram[:, :, nq * 512:nq * 512 + 512]
                              .rearrange("h p q -> p h q"), 2)
                if idx >= 1:
                    cqueue.extend(make_strip(order[idx - 1]))
                for h in range(H_L):
                    kh = h // 4
                    nkv = 4 * (qi + 1)
                    pes = []
                    for kvt in range(nkv):
                        # columns q < r*128 of a diagonal tile are fully masked
                        r = kvt - 4 * qi
                        c0 = max(r, 0) * 128
                        ps = ps_big.tile([128, 512], f32, tag="big", name="pss")
                        nc.tensor.matmul(
                            ps[:, c0:], kt_w[kvt // 4][:, kh,
                                               (kvt % 4) * 128:(kvt % 4 + 1) * 128],
                            qt[:, h, c0:], start=True, stop=True)
                        if r >= 0:
                            nc.vector.tensor_add(ps[:, c0:], ps[:, c0:],
                                                 mask_sb[:, r, c0:])
                        pe = exp_pool.tile([128, 512], bf16, tag="exp", name="pe")
                        nc.scalar.activation(pe[:, c0:], ps[:, c0:], Exp)
                        pes.append(pe)
                        if kvt == 3:
                            flush_pending()
                    # PE filler while ScalarE computes this head's exps
                    for _ in range(5):
                        if cqueue:
                            cqueue.pop(0)()
                    flush_pending()
                    aps = [ps_att.tile([128, 129], f32, tag="att", name=f"att{_qc}")
                           for _qc in range(4)]
                    for kvt in range(nkv):
                        for qc in range(4):
                            if qc < kvt - 4 * qi:
                                continue  # q-chunk entirely masked for this kv
                            nc.tensor.matmul(
                                aps[qc], pes[kvt][:, qc * 128:(qc + 1) * 128],
                                v_w[kvt // 4][:, kvt % 4, kh, 0:129],
                                start=(kvt == 0), stop=(kvt == 4 * qi + qc))
                    asbs = []
                    for qc in range(4):
                        rec = rec_pool.tile([128, 1], f32, tag="rec", name="rec")
                        nc.vector.reciprocal(rec, aps[qc][:, 128:129])
                        asb = asb_pool.tile([128, 128], bf16, tag="asb", name="asb")
                        nc.vector.tensor_scalar_mul(asb, aps[qc][:, 0:128], rec)
                        asbs.append(asb)

                    def defer(h=h, q0=q0, asbs=asbs):
                        for qc in range(4):
                            pst = ps_att.tile([128, 128], bf16, tag="att",
                                              name="pst")
                            nc.tensor.transpose(pst, asbs[qc], ident)
                            nc.vector.tensor_copy(
                                at_sb[:, h, q0 + qc * 128:q0 + (qc + 1) * 128],
                                pst)
                    pending.append(defer)
                flush_pending()
            cqueue.extend(make_strip(order[-1]))
            for c in cqueue:
                c()

    nc.finalize()
    return nc


def _prep_inputs(x, wq, wk, wv, wo, freqs_cos, freqs_sin):
    """Host-side shard prep. Returns in_maps for cores 0..7."""
    bf = ml_dtypes.bfloat16
    perm = np.concatenate([np.arange(0, HD, 2), np.arange(1, HD, 2)])  # rotate-half

    wq_p = (wq.astype(np.float32) / np.sqrt(HD)).reshape(DIM, N_HEADS, HD)[:, :, perm]
    wk_p = wk.astype(np.float32).reshape(DIM, N_KV, HD)[:, :, perm]

    cosT = np.ascontiguousarray(freqs_cos.astype(np.float32).T)  # [64, SEQ]
    sinT = np.ascontiguousarray(freqs_sin.astype(np.float32).T)

    # causal band mask tiles: maskb[kvl, r, ql] = 0 if r*128+kvl <= ql else NEG
    kvl = np.arange(128)[:, None, None]
    r = np.arange(4)[None, :, None]
    ql = np.arange(512)[None, None, :]
    maskb = np.where(r * 128 + kvl <= ql, 0.0, NEG).astype(np.float32)

    xTs = [np.ascontiguousarray(x[b].astype(np.float32).T).astype(bf)
           for b in range(BSZ)]

    in_maps = []
    for c in range(N_CORES):
        b, g = c // 4, c % 4
        in_maps.append({
            "xT": xTs[b],
            "wq": np.ascontiguousarray(
                wq_p[:, g * H_L:(g + 1) * H_L].reshape(DIM, HL)).astype(bf),
            "wk": np.ascontiguousarray(
                wk_p[:, g * KV_L:(g + 1) * KV_L].reshape(DIM, KV_L * HD)).astype(bf),
            "wv": np.ascontiguousarray(
                wv[:, g * KV_L * HD:(g + 1) * KV_L * HD]).astype(bf),
            "wo": np.ascontiguousarray(
                wo[g * HL:(g + 1) * HL]).astype(bf),
            "cosT": cosT,
            "sinT": sinT,
            "maskb": maskb,
        })
    return in_maps


def _run(inputs, trace=False):
    from concourse.bass_utils import run_bass_kernel_spmd

    if "nc" not in _cache:
        _cache["nc"] = _build()
    nc = _cache["nc"]

    in_maps = _prep_inputs(
        np.asarray(inputs["x"]), np.asarray(inputs["wq"]),
        np.asarray(inputs["wk"]), np.asarray(inputs["wv"]),
        np.asarray(inputs["wo"]), np.asarray(inputs["freqs_cos"]),
        np.asarray(inputs["freqs_sin"]))

    res = run_bass_kernel_spmd(nc, in_maps, core_ids=list(range(N_CORES)),
                               trace=trace)
    out = np.zeros((BSZ, SEQ, DIM), np.float32)
    for c in range(N_CORES):
        out[c // 4] += res.results[c]["out"].astype(np.float32)
    return out, res


def kernel(**inputs) -> np.ndarray:
    out, _ = _run(inputs, trace=False)
    return out


# revision 27
# speedup vs baseline: 1.0001x; 1.0001x over previous
"""Distributed GQA attention prefill kernel for 8 TRN2 NeuronCores.

Problem: llama-style attention, BSZ=2, SEQ=2048, DIM=4096, 32 Q heads,
8 KV heads, head_dim=128, causal prefill (start_pos=0, caches zero).

Sharding: data-parallel over batch (2) x tensor-parallel over heads (4).
Core c = (b, g) with b = c // 4, g = c % 4 handles batch b, Q heads
8g..8g+7, KV heads 2g..2g+1, and wo rows 1024g..1024(g+1). Each core
emits a partial [2048, 4096] output; the host sums the 4 TP partials
per batch. No collectives.

On-chip layout trick: everything is computed in "transposed" layouts so
no activation transpose is ever needed:
  QT[d, t] = wq.T @ x.T       (lhsT = wq natural, rhs = xT from host)
  KT[d, t] = wk.T @ x.T
  V[t, d]  = x @ wv           (lhsT = xT chunk, rhs = wv natural)
  scoresT[kv, q] = K @ QT     (lhsT = KT tile, rhs = QT tile)
  attn[q, d+1]   = P @ [V|1]  (lhsT = expT tile, rhs = V with ones col
                               -> last column accumulates the softmax
                               denominator for free)
RoPE is applied in rotate-half form: the head_dim of wq/wk is permuted
on the host (even dims first, odd dims second) which leaves all dot
products unchanged; cos/sin arrive transposed [64, t].

Schedule: stage A does Q/K/V projection in one x-pass (QT spilled to
DRAM); stage B (attention) then runs with stage C (output projection)
interleaved at head granularity — C's matmuls fill the PE while ScalarE
computes the next head's exp row.
"""

import sys

for p in ("/opt/pypackages", "/opt/trn_rl_repo"):
    if p not in sys.path:
        sys.path.insert(0, p)

import numpy as np
import ml_dtypes

BSZ, SEQ, DIM = 2, 2048, 4096
N_HEADS, N_KV, HD = 32, 8, 128
H_L, KV_L = 8, 2          # per-core local Q heads / KV heads
HL = H_L * HD             # 1024 local head dims
N_CORES = 8
WIN = 256                 # stage-A token window
NW = SEQ // WIN
NK = DIM // 128
NQT = SEQ // 512          # attention q-tiles
NEG = -1e9

_cache = {}


def _build():
    import concourse.mybir as mybir
    import concourse.tile as tile
    from concourse import bacc
    from concourse.masks import make_identity
    from contextlib import ExitStack

    f32 = mybir.dt.float32
    bf16 = mybir.dt.bfloat16
    Exp = mybir.ActivationFunctionType.Exp

    nc = bacc.Bacc()
    xT = nc.declare_dram_parameter("xT", [DIM, SEQ], bf16, isOutput=False)
    wq = nc.declare_dram_parameter("wq", [DIM, HL], bf16, isOutput=False)
    wk = nc.declare_dram_parameter("wk", [DIM, KV_L * HD], bf16, isOutput=False)
    wv = nc.declare_dram_parameter("wv", [DIM, KV_L * HD], bf16, isOutput=False)
    wo = nc.declare_dram_parameter("wo", [HL, DIM], bf16, isOutput=False)
    cosT = nc.declare_dram_parameter("cosT", [64, SEQ], f32, isOutput=False)
    sinT = nc.declare_dram_parameter("sinT", [64, SEQ], f32, isOutput=False)
    maskb = nc.declare_dram_parameter("maskb", [128, 4, 512], f32, isOutput=False)
    out = nc.declare_dram_parameter("out", [SEQ, DIM], f32, isOutput=True)

    qt_dram = nc.dram_tensor("qt_spill", [H_L, HD, SEQ], bf16)

    def dma_split(dst, src, n):
        """Issue n parallel DMAs over the ko axis (dim 1 of dst)."""
        ko = dst.shape[1]
        step = ko // n
        for i in range(n):
            nc.sync.dma_start(
                out=dst[:, i * step:(i + 1) * step],
                in_=src[:, i * step:(i + 1) * step])

    with tile.TileContext(nc) as tc, ExitStack() as res:
        ps_big = res.enter_context(tc.tile_pool(name="ps_big", bufs=4, space="PSUM"))
        ps_att = res.enter_context(tc.tile_pool(name="ps_att", bufs=4, space="PSUM"))
        resid = res.enter_context(tc.tile_pool(name="resid", bufs=1))
        qt_pool = res.enter_context(tc.tile_pool(name="qt", bufs=2))
        qts = {}

        # per-window K/V tiles (window = 512 tokens for attention indexing)
        kt_w = [resid.tile([128, KV_L, 512], bf16, tag=f"kt{w}", name=f"kt{w}")
                for w in range(NQT)]
        v_w = [resid.tile([128, 4, KV_L, 130], bf16, tag=f"v{w}",
                          name=f"v{w}") for w in range(NQT)]
        ident = resid.tile([128, 128], bf16, tag="ident")
        cos_sb = resid.tile([64, SEQ], f32, tag="cos")
        sin_sb = resid.tile([64, SEQ], f32, tag="sin")
        mask_sb = resid.tile([128, 4, 512], f32, tag="mask")

        def rope(ps, dst, t0, tw):
            """dst[0:64]=e*c-o*s ; dst[64:128]=e*s+o*c (e=ps[0:64], o=ps[64:128])."""
            c = cos_sb[:, t0:t0 + tw]
            s = sin_sb[:, t0:t0 + tw]
            t1 = rope_pool.tile([64, WIN], f32, tag="r1", name="r1")[:, :tw]
            t2 = rope_pool.tile([64, WIN], f32, tag="r2", name="r2")[:, :tw]
            nc.vector.tensor_mul(t1, ps[0:64, :tw], c)
            nc.vector.tensor_mul(t2, ps[64:128, :tw], s)
            nc.vector.tensor_sub(dst[0:64, :tw], t1, t2)
            t3 = rope_pool.tile([64, WIN], f32, tag="r1", name="r3")[:, :tw]
            t4 = rope_pool.tile([64, WIN], f32, tag="r2", name="r4")[:, :tw]
            nc.vector.tensor_mul(t3, ps[0:64, :tw], s)
            nc.vector.tensor_mul(t4, ps[64:128, :tw], c)
            nc.vector.tensor_add(dst[64:128, :tw], t3, t4)

        # ---- stage A: Q/K/V projection + RoPE in one x-pass ---------------
        with ExitStack() as sa:
            wq_sb = sa.enter_context(tc.tile_pool(name="wq", bufs=1)).tile(
                [128, NK, HL], bf16, tag="wq")
            wk_sb = sa.enter_context(tc.tile_pool(name="wk", bufs=1)).tile(
                [128, NK, KV_L * HD], bf16, tag="wk")
            wv_sb = sa.enter_context(tc.tile_pool(name="wv", bufs=1)).tile(
                [128, NK, KV_L * HD], bf16, tag="wv")
            xt_pool = sa.enter_context(tc.tile_pool(name="xt", bufs=2))
            rope_pool = sa.enter_context(tc.tile_pool(name="rope", bufs=2))
            qsp_pool = sa.enter_context(tc.tile_pool(name="qsp", bufs=2))

            # critical-path loads first: window-0 xT + wk get the DMA queues
            # to themselves so the first matmul starts in ~2us; wq's 8MB is
            # emitted after window 0's K/V work (Q is last in the window).
            # tiny first chunks so the very first matmul's operands land fast
            xt0 = xt_pool.tile([128, NK, WIN], bf16, tag="xt", name="xt0")
            xt0_src = xT[:, 0:WIN].rearrange("(ko p) t -> p ko t", p=128)
            wk_src = wk.rearrange("(ko p) d -> p ko d", p=128)
            nc.sync.dma_start(out=xt0[:, 0:2], in_=xt0_src[:, 0:2])
            nc.sync.dma_start(out=wk_sb[:, 0:2], in_=wk_src[:, 0:2])
            dma_split(xt0[:, 2:], xt0_src[:, 2:], 6)
            dma_split(wk_sb[:, 2:], wk_src[:, 2:], 6)
            dma_split(wv_sb, wv.rearrange("(ko p) d -> p ko d", p=128), 8)
            nc.sync.dma_start(out=cos_sb, in_=cosT[:, :])
            nc.sync.dma_start(out=sin_sb, in_=sinT[:, :])
            make_identity(nc, ident)
            for w_ in range(NQT):
                nc.vector.memset(v_w[w_][:, :, :, 128:129], 1.0)

            for w in range(NW):
                t0 = w * WIN
                wa, wo512 = t0 // 512, (t0 % 512)
                if w == 0:
                    xt = xt0
                else:
                    xt = xt_pool.tile([128, NK, WIN], bf16, tag="xt", name="xt")
                    dma_split(xt, xT[:, t0:t0 + WIN].rearrange(
                        "(ko p) t -> p ko t", p=128), 4)
                for kh in range(KV_L):
                    ps = ps_big.tile([128, 512], f32, tag="big", name="psk")
                    for k in range(NK):
                        nc.tensor.matmul(
                            ps[:, :WIN], wk_sb[:, k, kh * HD:(kh + 1) * HD],
                            xt[:, k], start=(k == 0), stop=(k == NK - 1))
                    rope(ps, kt_w[wa][:, kh, wo512:wo512 + WIN], t0, WIN)
                for tc_ in range(WIN // 128):
                    ps = ps_big.tile([128, 512], f32, tag="big", name="psv")
                    for k in range(NK):
                        nc.tensor.matmul(
                            ps[:, :KV_L * HD], xt[:, k, tc_ * 128:(tc_ + 1) * 128],
                            wv_sb[:, k], start=(k == 0), stop=(k == NK - 1))
                    for kh in range(KV_L):
                        nc.scalar.copy(
                            v_w[wa][:, wo512 // 128 + tc_, kh, 0:128],
                            ps[:, kh * HD:(kh + 1) * HD])
                if w == 0:
                    dma_split(wq_sb, wq.rearrange("(ko p) d -> p ko d", p=128), 8)
                    nc.sync.dma_start(out=mask_sb, in_=maskb[:, :, :])
                for h in range(H_L):
                    ps = ps_big.tile([128, 512], f32, tag="big", name="psq")
                    for k in range(NK):
                        nc.tensor.matmul(
                            ps[:, :WIN], wq_sb[:, k, h * HD:(h + 1) * HD],
                            xt[:, k], start=(k == 0), stop=(k == NK - 1))
                    qs = qsp_pool.tile([128, WIN], bf16, tag="qs", name="qs")
                    rope(ps, qs, t0, WIN)
                    nc.sync.dma_start(out=qt_dram[h, :, t0:t0 + WIN], in_=qs)
                if t0 + WIN == 1024:
                    # first attention q-tile (q1) fully spilled -> prefetch it
                    qts[1] = qt_pool.tile([128, H_L, 512], bf16, tag="qt",
                                          name="qt0")
                    dma_split(qts[1], qt_dram[:, :, 512:1024]
                              .rearrange("h p q -> p h q"), 2)

        # ---- stage B (attention) with stage C (out-proj) interleaved ------
        with ExitStack() as bc:
            exp_pool = bc.enter_context(tc.tile_pool(name="exp", bufs=8))
            asb_pool = bc.enter_context(tc.tile_pool(name="asb", bufs=8))
            rec_pool = bc.enter_context(tc.tile_pool(name="rec", bufs=8))
            at_sb = bc.enter_context(tc.tile_pool(name="at", bufs=1)).tile(
                [128, H_L, SEQ], bf16, tag="at")
            wo_pool = bc.enter_context(tc.tile_pool(name="wo", bufs=2))
            out_pool = bc.enter_context(tc.tile_pool(name="outp", bufs=4))

            pending = []

            def flush_pending():
                while pending:
                    pending.pop(0)()

            wo_cur = [None]

            def make_strip(qs_):
                """Emission closures for out-proj of token strip qs_ (4 ti)."""
                cls = []
                for di in range(DIM // 512):
                    def load_wo(di=di):
                        wot = wo_pool.tile([128, H_L, 512], bf16, tag="wo",
                                           name="wot")
                        dma_split(wot, wo[:, di * 512:(di + 1) * 512].rearrange(
                            "(ho p) d -> p ho d", p=128), 2)
                        wo_cur[0] = wot
                    cls.append(load_wo)
                    for tj in range(4):
                        def pair(di=di, ti=qs_ * 4 + tj):
                            wot = wo_cur[0]
                            ps = ps_big.tile([128, 512], f32, tag="big",
                                             name="pso")
                            for half in range(2):
                                hs = slice(half * 256, (half + 1) * 256)
                                for ho in range(H_L):
                                    nc.tensor.matmul(
                                        ps[:, hs],
                                        at_sb[:, ho, ti * 128:(ti + 1) * 128],
                                        wot[:, ho, hs], start=(ho == 0),
                                        stop=(ho == H_L - 1))
                            osb = out_pool.tile([128, 512], f32, tag="osb",
                                                name="osb")
                            nc.vector.tensor_copy(osb, ps)
                            nc.sync.dma_start(
                                out=out[ti * 128:(ti + 1) * 128,
                                        di * 512:(di + 1) * 512],
                                in_=osb)
                        cls.append(pair)
                return cls

            cqueue = []

            # q1 first so its finished strip feeds PE fillers during B(q0)
            order = [1, 0, 2, 3]

            for idx, qi in enumerate(order):
                q0 = qi * 512
                qt = qts.pop(qi)
                if idx + 1 < len(order):
                    nq = order[idx + 1]
                    qts[nq] = qt_pool.tile([128, H_L, 512], bf16,
                                           tag="qt", name="qtn")
                    dma_split(qts[nq], qt_dram[:, :, nq * 512:nq * 512 + 512]
                              .rearrange("h p q -> p h q"), 2)
                if idx >= 1:
                    cqueue.extend(make_strip(order[idx - 1]))
                for h in range(H_L):
                    kh = h // 4
                    nkv = 4 * (qi + 1)
                    pes = []
                    for kvt in range(nkv):
                        # columns q < r*128 of a diagonal tile are fully masked
                        r = kvt - 4 * qi
                        c0 = max(r, 0) * 128
                        ps = ps_big.tile([128, 512], f32, tag="big", name="pss")
                        kt_tile = kt_w[kvt // 4][:, kh,
                                                 (kvt % 4) * 128:(kvt % 4 + 1) * 128]
                        if c0 == 0:
                            # two N=256 halves stream slightly faster than one 512
                            nc.tensor.matmul(ps[:, 0:256], kt_tile,
                                             qt[:, h, 0:256], start=True, stop=True)
                            nc.tensor.matmul(ps[:, 256:512], kt_tile,
                                             qt[:, h, 256:512], start=True, stop=True)
                        else:
                            nc.tensor.matmul(ps[:, c0:], kt_tile,
                                             qt[:, h, c0:], start=True, stop=True)
                        if r >= 0:
                            nc.vector.tensor_add(ps[:, c0:], ps[:, c0:],
                                                 mask_sb[:, r, c0:])
                        pe = exp_pool.tile([128, 512], bf16, tag="exp", name="pe")
                        nc.scalar.activation(pe[:, c0:], ps[:, c0:], Exp)
                        pes.append(pe)
                        if kvt == 3:
                            flush_pending()
                    # PE filler while ScalarE computes this head's exps
                    for _ in range(5):
                        if cqueue:
                            cqueue.pop(0)()
                    flush_pending()
                    aps = [ps_att.tile([128, 129], f32, tag="att", name=f"att{_qc}")
                           for _qc in range(4)]
                    for kvt in range(nkv):
                        for qc in range(4):
                            if qc < kvt - 4 * qi:
                                continue  # q-chunk entirely masked for this kv
                            nc.tensor.matmul(
                                aps[qc], pes[kvt][:, qc * 128:(qc + 1) * 128],
                                v_w[kvt // 4][:, kvt % 4, kh, 0:129],
                                start=(kvt == 0), stop=(kvt == 4 * qi + qc))
                    asbs = []
                    for qc in range(4):
                        rec = rec_pool.tile([128, 1], f32, tag="rec", name="rec")
                        nc.vector.reciprocal(rec, aps[qc][:, 128:129])
                        asb = asb_pool.tile([128, 128], bf16, tag="asb", name="asb")
                        nc.vector.tensor_scalar_mul(asb, aps[qc][:, 0:128], rec)
                        asbs.append(asb)

                    def defer(h=h, q0=q0, asbs=asbs):
                        for qc in range(4):
                            pst = ps_att.tile([128, 128], bf16, tag="att",
                                              name="pst")
                            nc.tensor.transpose(pst, asbs[qc], ident)
                            nc.vector.tensor_copy(
                                at_sb[:, h, q0 + qc * 128:q0 + (qc + 1) * 128],
                                pst)
                    pending.append(defer)
                flush_pending()
            cqueue.extend(make_strip(order[-1]))
            for c in cqueue:
                c()

    nc.finalize()
    return nc


def _prep_inputs(x, wq, wk, wv, wo, freqs_cos, freqs_sin):
    """Host-side shard prep. Returns in_maps for cores 0..7."""
    bf = ml_dtypes.bfloat16
    perm = np.concatenate([np.arange(0, HD, 2), np.arange(1, HD, 2)])  # rotate-half

    wq_p = (wq.astype(np.float32) / np.sqrt(HD)).reshape(DIM, N_HEADS, HD)[:, :, perm]
    wk_p = wk.astype(np.float32).reshape(DIM, N_KV, HD)[:, :, perm]

    cosT = np.ascontiguousarray(freqs_cos.astype(np.float32).T)  # [64, SEQ]
    sinT = np.ascontiguousarray(freqs_sin.astype(np.float32).T)

    # causal band mask tiles: maskb[kvl, r, ql] = 0 if r*128+kvl <= ql else NEG
    kvl = np.arange(128)[:, None, None]
    r = np.arange(4)[None, :, None]
    ql = np.arange(512)[None, None, :]
    maskb = np.where(r * 128 + kvl <= ql, 0.0, NEG).astype(np.float32)

    xTs = [np.ascontiguousarray(x[b].astype(np.float32).T).astype(bf)
           for b in range(BSZ)]

    in_maps = []
    for c in range(N_CORES):
        b, g = c // 4, c % 4
        in_maps.append({
            "xT": xTs[b],
            "wq": np.ascontiguousarray(
                wq_p[:, g * H_L:(g + 1) * H_L].reshape(DIM, HL)).astype(bf),
            "wk": np.ascontiguousarray(
                wk_p[:, g * KV_L:(g + 1) * KV_L].reshape(DIM, KV_L * HD)).astype(bf),
            "wv": np.ascontiguousarray(
                wv[:, g * KV_L * HD:(g + 1) * KV_L * HD]).astype(bf),
            "wo": np.ascontiguousarray(
                wo[g * HL:(g + 1) * HL]).astype(bf),
            "cosT": cosT,
            "sinT": sinT,
            "maskb": maskb,
        })
    return in_maps


def _run(inputs, trace=False):
    from concourse.bass_utils import run_bass_kernel_spmd

    if "nc" not in _cache:
        _cache["nc"] = _build()
    nc = _cache["nc"]

    in_maps = _prep_inputs(
        np.asarray(inputs["x"]), np.asarray(inputs["wq"]),
        np.asarray(inputs["wk"]), np.asarray(inputs["wv"]),
        np.asarray(inputs["wo"]), np.asarray(inputs["freqs_cos"]),
        np.asarray(inputs["freqs_sin"]))

    res = run_bass_kernel_spmd(nc, in_maps, core_ids=list(range(N_CORES)),
                               trace=trace)
    out = np.zeros((BSZ, SEQ, DIM), np.float32)
    for c in range(N_CORES):
        out[c // 4] += res.results[c]["out"].astype(np.float32)
    return out, res


def kernel(**inputs) -> np.ndarray:
    out, _ = _run(inputs, trace=False)
    return out


# revision 28
# speedup vs baseline: 1.0023x; 1.0022x over previous
"""Distributed GQA attention prefill kernel for 8 TRN2 NeuronCores.

Problem: llama-style attention, BSZ=2, SEQ=2048, DIM=4096, 32 Q heads,
8 KV heads, head_dim=128, causal prefill (start_pos=0, caches zero).

Sharding: data-parallel over batch (2) x tensor-parallel over heads (4).
Core c = (b, g) with b = c // 4, g = c % 4 handles batch b, Q heads
8g..8g+7, KV heads 2g..2g+1, and wo rows 1024g..1024(g+1). Each core
emits a partial [2048, 4096] output; the host sums the 4 TP partials
per batch. No collectives.

On-chip layout trick: everything is computed in "transposed" layouts so
no activation transpose is ever needed:
  QT[d, t] = wq.T @ x.T       (lhsT = wq natural, rhs = xT from host)
  KT[d, t] = wk.T @ x.T
  V[t, d]  = x @ wv           (lhsT = xT chunk, rhs = wv natural)
  scoresT[kv, q] = K @ QT     (lhsT = KT tile, rhs = QT tile)
  attn[q, d+1]   = P @ [V|1]  (lhsT = expT tile, rhs = V with ones col
                               -> last column accumulates the softmax
                               denominator for free)
RoPE is applied in rotate-half form: the head_dim of wq/wk is permuted
on the host (even dims first, odd dims second) which leaves all dot
products unchanged; cos/sin arrive transposed [64, t].

Schedule: stage A does Q/K/V projection in one x-pass (QT spilled to
DRAM); stage B (attention) then runs with stage C (output projection)
interleaved at head granularity — C's matmuls fill the PE while ScalarE
computes the next head's exp row.
"""

import sys

for p in ("/opt/pypackages", "/opt/trn_rl_repo"):
    if p not in sys.path:
        sys.path.insert(0, p)

import numpy as np
import ml_dtypes

BSZ, SEQ, DIM = 2, 2048, 4096
N_HEADS, N_KV, HD = 32, 8, 128
H_L, KV_L = 8, 2          # per-core local Q heads / KV heads
HL = H_L * HD             # 1024 local head dims
N_CORES = 8
WIN = 256                 # stage-A token window
NW = SEQ // WIN
NK = DIM // 128
NQT = SEQ // 512          # attention q-tiles
NEG = -1e9

_cache = {}


def _build():
    import concourse.mybir as mybir
    import concourse.tile as tile
    from concourse import bacc
    from concourse.masks import make_identity
    from contextlib import ExitStack

    f32 = mybir.dt.float32
    bf16 = mybir.dt.bfloat16
    Exp = mybir.ActivationFunctionType.Exp

    nc = bacc.Bacc()
    xT = nc.declare_dram_parameter("xT", [DIM, SEQ], bf16, isOutput=False)
    wq = nc.declare_dram_parameter("wq", [DIM, HL], bf16, isOutput=False)
    wk = nc.declare_dram_parameter("wk", [DIM, KV_L * HD], bf16, isOutput=False)
    wv = nc.declare_dram_parameter("wv", [DIM, KV_L * HD], bf16, isOutput=False)
    wo = nc.declare_dram_parameter("wo", [HL, DIM], bf16, isOutput=False)
    cosT = nc.declare_dram_parameter("cosT", [64, SEQ], f32, isOutput=False)
    sinT = nc.declare_dram_parameter("sinT", [64, SEQ], f32, isOutput=False)
    maskb = nc.declare_dram_parameter("maskb", [128, 4, 512], f32, isOutput=False)
    out = nc.declare_dram_parameter("out", [SEQ, DIM], f32, isOutput=True)

    qt_dram = nc.dram_tensor("qt_spill", [H_L, HD, SEQ], bf16)

    def dma_split(dst, src, n):
        """Issue n parallel DMAs over the ko axis (dim 1 of dst)."""
        ko = dst.shape[1]
        step = ko // n
        for i in range(n):
            nc.sync.dma_start(
                out=dst[:, i * step:(i + 1) * step],
                in_=src[:, i * step:(i + 1) * step])

    with tile.TileContext(nc) as tc, ExitStack() as res:
        ps_big = res.enter_context(tc.tile_pool(name="ps_big", bufs=4, space="PSUM"))
        ps_att = res.enter_context(tc.tile_pool(name="ps_att", bufs=4, space="PSUM"))
        resid = res.enter_context(tc.tile_pool(name="resid", bufs=1))
        qt_pool = res.enter_context(tc.tile_pool(name="qt", bufs=2))
        qts = {}

        # per-window K/V tiles (window = 512 tokens for attention indexing)
        kt_w = [resid.tile([128, KV_L, 512], bf16, tag=f"kt{w}", name=f"kt{w}")
                for w in range(NQT)]
        v_w = [resid.tile([128, 4, KV_L, 130], bf16, tag=f"v{w}",
                          name=f"v{w}") for w in range(NQT)]
        ident = resid.tile([128, 128], bf16, tag="ident")
        cos_sb = resid.tile([64, SEQ], f32, tag="cos")
        sin_sb = resid.tile([64, SEQ], f32, tag="sin")
        mask_sb = resid.tile([128, 4, 512], f32, tag="mask")

        def rope(ps, dst, t0, tw):
            """dst[0:64]=e*c-o*s ; dst[64:128]=e*s+o*c (e=ps[0:64], o=ps[64:128])."""
            c = cos_sb[:, t0:t0 + tw]
            s = sin_sb[:, t0:t0 + tw]
            t1 = rope_pool.tile([64, WIN], f32, tag="r1", name="r1")[:, :tw]
            t2 = rope_pool.tile([64, WIN], f32, tag="r2", name="r2")[:, :tw]
            nc.vector.tensor_mul(t1, ps[0:64, :tw], c)
            nc.vector.tensor_mul(t2, ps[64:128, :tw], s)
            nc.vector.tensor_sub(dst[0:64, :tw], t1, t2)
            t3 = rope_pool.tile([64, WIN], f32, tag="r1", name="r3")[:, :tw]
            t4 = rope_pool.tile([64, WIN], f32, tag="r2", name="r4")[:, :tw]
            nc.vector.tensor_mul(t3, ps[0:64, :tw], s)
            nc.vector.tensor_mul(t4, ps[64:128, :tw], c)
            nc.vector.tensor_add(dst[64:128, :tw], t3, t4)

        # ---- stage A: Q/K/V projection + RoPE in one x-pass ---------------
        with ExitStack() as sa:
            wq_sb = sa.enter_context(tc.tile_pool(name="wq", bufs=1)).tile(
                [128, NK, HL], bf16, tag="wq")
            wk_sb = sa.enter_context(tc.tile_pool(name="wk", bufs=1)).tile(
                [128, NK, KV_L * HD], bf16, tag="wk")
            wv_sb = sa.enter_context(tc.tile_pool(name="wv", bufs=1)).tile(
                [128, NK, KV_L * HD], bf16, tag="wv")
            xt_pool = sa.enter_context(tc.tile_pool(name="xt", bufs=2))
            rope_pool = sa.enter_context(tc.tile_pool(name="rope", bufs=2))
            qsp_pool = sa.enter_context(tc.tile_pool(name="qsp", bufs=2))

            # critical-path loads first: window-0 xT + wk get the DMA queues
            # to themselves so the first matmul starts in ~2us; wq's 8MB is
            # emitted after window 0's K/V work (Q is last in the window).
            # tiny first chunks so the very first matmul's operands land fast
            xt0 = xt_pool.tile([128, NK, WIN], bf16, tag="xt", name="xt0")
            xt0_src = xT[:, 0:WIN].rearrange("(ko p) t -> p ko t", p=128)
            wk_src = wk.rearrange("(ko p) d -> p ko d", p=128)
            nc.sync.dma_start(out=xt0[:, 0:2], in_=xt0_src[:, 0:2])
            nc.sync.dma_start(out=wk_sb[:, 0:2], in_=wk_src[:, 0:2])
            dma_split(xt0[:, 2:], xt0_src[:, 2:], 6)
            dma_split(wk_sb[:, 2:], wk_src[:, 2:], 6)
            dma_split(wv_sb, wv.rearrange("(ko p) d -> p ko d", p=128), 8)
            nc.sync.dma_start(out=cos_sb, in_=cosT[:, :])
            nc.sync.dma_start(out=sin_sb, in_=sinT[:, :])
            make_identity(nc, ident)
            for w_ in range(NQT):
                nc.vector.memset(v_w[w_][:, :, :, 128:129], 1.0)

            for w in range(NW):
                t0 = w * WIN
                wa, wo512 = t0 // 512, (t0 % 512)
                if w == 0:
                    xt = xt0
                else:
                    xt = xt_pool.tile([128, NK, WIN], bf16, tag="xt", name="xt")
                    dma_split(xt, xT[:, t0:t0 + WIN].rearrange(
                        "(ko p) t -> p ko t", p=128), 4)
                for kh in range(KV_L):
                    ps = ps_big.tile([128, 512], f32, tag="big", name="psk")
                    for k in range(NK):
                        nc.tensor.matmul(
                            ps[:, :WIN], wk_sb[:, k, kh * HD:(kh + 1) * HD],
                            xt[:, k], start=(k == 0), stop=(k == NK - 1))
                    rope(ps, kt_w[wa][:, kh, wo512:wo512 + WIN], t0, WIN)
                for tc_ in range(WIN // 128):
                    ps = ps_big.tile([128, 512], f32, tag="big", name="psv")
                    for k in range(NK):
                        nc.tensor.matmul(
                            ps[:, :KV_L * HD], xt[:, k, tc_ * 128:(tc_ + 1) * 128],
                            wv_sb[:, k], start=(k == 0), stop=(k == NK - 1))
                    for kh in range(KV_L):
                        nc.scalar.copy(
                            v_w[wa][:, wo512 // 128 + tc_, kh, 0:128],
                            ps[:, kh * HD:(kh + 1) * HD])
                if w == 0:
                    dma_split(wq_sb, wq.rearrange("(ko p) d -> p ko d", p=128), 8)
                    nc.sync.dma_start(out=mask_sb, in_=maskb[:, :, :])
                for h in range(H_L):
                    ps = ps_big.tile([128, 512], f32, tag="big", name="psq")
                    for k in range(NK):
                        nc.tensor.matmul(
                            ps[:, :WIN], wq_sb[:, k, h * HD:(h + 1) * HD],
                            xt[:, k], start=(k == 0), stop=(k == NK - 1))
                    qs = qsp_pool.tile([128, WIN], bf16, tag="qs", name="qs")
                    rope(ps, qs, t0, WIN)
                    nc.sync.dma_start(out=qt_dram[h, :, t0:t0 + WIN], in_=qs)
                if t0 + WIN == 1024:
                    # first attention q-tile (q1) fully spilled -> prefetch it
                    qts[1] = qt_pool.tile([128, H_L, 512], bf16, tag="qt",
                                          name="qt0")
                    dma_split(qts[1], qt_dram[:, :, 512:1024]
                              .rearrange("h p q -> p h q"), 2)

        # ---- stage B (attention) with stage C (out-proj) interleaved ------
        with ExitStack() as bc:
            exp_pool = bc.enter_context(tc.tile_pool(name="exp", bufs=8))
            asb_pool = bc.enter_context(tc.tile_pool(name="asb", bufs=8))
            rec_pool = bc.enter_context(tc.tile_pool(name="rec", bufs=8))
            at_sb = bc.enter_context(tc.tile_pool(name="at", bufs=1)).tile(
                [128, H_L, SEQ], bf16, tag="at")
            wo_pool = bc.enter_context(tc.tile_pool(name="wo", bufs=2))
            out_pool = bc.enter_context(tc.tile_pool(name="outp", bufs=4))

            pending = []

            def flush_pending():
                while pending:
                    pending.pop(0)()

            wo_cur = [None]

            def make_strip(qs_):
                """Emission closures for out-proj of token strip qs_ (4 ti)."""
                cls = []
                for di in range(DIM // 512):
                    def load_wo(di=di):
                        wot = wo_pool.tile([128, H_L, 512], bf16, tag="wo",
                                           name="wot")
                        dma_split(wot, wo[:, di * 512:(di + 1) * 512].rearrange(
                            "(ho p) d -> p ho d", p=128), 2)
                        wo_cur[0] = wot
                    cls.append(load_wo)
                    for tj in range(4):
                        def pair(di=di, ti=qs_ * 4 + tj):
                            wot = wo_cur[0]
                            ps = ps_big.tile([128, 512], f32, tag="big",
                                             name="pso")
                            for ho in range(H_L):
                                nc.tensor.matmul(
                                    ps, at_sb[:, ho, ti * 128:(ti + 1) * 128],
                                    wot[:, ho], start=(ho == 0),
                                    stop=(ho == H_L - 1))
                            osb = out_pool.tile([128, 512], f32, tag="osb",
                                                name="osb")
                            nc.vector.tensor_copy(osb, ps)
                            nc.sync.dma_start(
                                out=out[ti * 128:(ti + 1) * 128,
                                        di * 512:(di + 1) * 512],
                                in_=osb)
                        cls.append(pair)
                return cls

            cqueue = []

            # q1 first so its finished strip feeds PE fillers during B(q0)
            order = [1, 0, 2, 3]

            for idx, qi in enumerate(order):
                q0 = qi * 512
                qt = qts.pop(qi)
                if idx + 1 < len(order):
                    nq = order[idx + 1]
                    qts[nq] = qt_pool.tile([128, H_L, 512], bf16,
                                           tag="qt", name="qtn")
                    dma_split(qts[nq], qt_dram[:, :, nq * 512:nq * 512 + 512]
                              .rearrange("h p q -> p h q"), 2)
                if idx >= 1:
                    cqueue.extend(make_strip(order[idx - 1]))
                for h in range(H_L):
                    kh = h // 4
                    nkv = 4 * (qi + 1)
                    pes = []
                    for kvt in range(nkv):
                        # columns q < r*128 of a diagonal tile are fully masked
                        r = kvt - 4 * qi
                        c0 = max(r, 0) * 128
                        ps = ps_big.tile([128, 512], f32, tag="big", name="pss")
                        nc.tensor.matmul(
                            ps[:, c0:], kt_w[kvt // 4][:, kh,
                                               (kvt % 4) * 128:(kvt % 4 + 1) * 128],
                            qt[:, h, c0:], start=True, stop=True)
                        if r >= 0:
                            nc.vector.tensor_add(ps[:, c0:], ps[:, c0:],
                                                 mask_sb[:, r, c0:])
                        pe = exp_pool.tile([128, 512], bf16, tag="exp", name="pe")
                        nc.scalar.activation(pe[:, c0:], ps[:, c0:], Exp)
                        pes.append(pe)
                        if kvt == 3:
                            flush_pending()
                    # PE filler while ScalarE computes this head's exps
                    for _ in range(5):
                        if cqueue:
                            cqueue.pop(0)()
                    flush_pending()
                    aps = [ps_att.tile([128, 129], f32, tag="att", name=f"att{_qc}")
                           for _qc in range(4)]
                    for kvt in range(nkv):
                        for qc in range(4):
                            if qc < kvt - 4 * qi:
                                continue  # q-chunk entirely masked for this kv
                            nc.tensor.matmul(
                                aps[qc], pes[kvt][:, qc * 128:(qc + 1) * 128],
                                v_w[kvt // 4][:, kvt % 4, kh, 0:129],
                                start=(kvt == 0), stop=(kvt == 4 * qi + qc))
                    asbs = []
                    for qc in range(4):
                        rec = rec_pool.tile([128, 1], f32, tag="rec", name="rec")
                        nc.vector.reciprocal(rec, aps[qc][:, 128:129])
                        asb = asb_pool.tile([128, 128], bf16, tag="asb", name="asb")
                        nc.vector.tensor_scalar_mul(asb, aps[qc][:, 0:128], rec)
                        asbs.append(asb)

                    def defer(h=h, q0=q0, asbs=asbs):
                        for qc in range(4):
                            pst = ps_att.tile([128, 128], bf16, tag="att",
                                              name="pst")
                            nc.tensor.transpose(pst, asbs[qc], ident)
                            nc.vector.tensor_copy(
                                at_sb[:, h, q0 + qc * 128:q0 + (qc + 1) * 128],
                                pst)
                    pending.append(defer)
                flush_pending()
            cqueue.extend(make_strip(order[-1]))
            for c in cqueue:
                c()

    nc.finalize()
    return nc


def _prep_inputs(x, wq, wk, wv, wo, freqs_cos, freqs_sin):
    """Host-side shard prep. Returns in_maps for cores 0..7."""
    bf = ml_dtypes.bfloat16
    perm = np.concatenate([np.arange(0, HD, 2), np.arange(1, HD, 2)])  # rotate-half

    wq_p = (wq.astype(np.float32) / np.sqrt(HD)).reshape(DIM, N_HEADS, HD)[:, :, perm]
    wk_p = wk.astype(np.float32).reshape(DIM, N_KV, HD)[:, :, perm]

    cosT = np.ascontiguousarray(freqs_cos.astype(np.float32).T)  # [64, SEQ]
    sinT = np.ascontiguousarray(freqs_sin.astype(np.float32).T)

    # causal band mask tiles: maskb[kvl, r, ql] = 0 if r*128+kvl <= ql else NEG
    kvl = np.arange(128)[:, None, None]
    r = np.arange(4)[None, :, None]
    ql = np.arange(512)[None, None, :]
    maskb = np.where(r * 128 + kvl <= ql, 0.0, NEG).astype(np.float32)

    xTs = [np.ascontiguousarray(x[b].astype(np.float32).T).astype(bf)
           for b in range(BSZ)]

    in_maps = []
    for c in range(N_CORES):
        b, g = c // 4, c % 4
        in_maps.append({
            "xT": xTs[b],
            "wq": np.ascontiguousarray(
                wq_p[:, g * H_L:(g + 1) * H_L].reshape(DIM, HL)).astype(bf),
            "wk": np.ascontiguousarray(
                wk_p[:, g * KV_L:(g + 1) * KV_L].reshape(DIM, KV_L * HD)).astype(bf),
            "wv": np.ascontiguousarray(
                wv[:, g * KV_L * HD:(g + 1) * KV_L * HD]).astype(bf),
            "wo": np.ascontiguousarray(
                wo[g * HL:(g + 1) * HL]).astype(bf),
            "cosT": cosT,
            "sinT": sinT,
            "maskb": maskb,
        })
    return in_maps


def _run(inputs, trace=False):
    from concourse.bass_utils import run_bass_kernel_spmd

    if "nc" not in _cache:
        _cache["nc"] = _build()
    nc = _cache["nc"]

    in_maps = _prep_inputs(
        np.asarray(inputs["x"]), np.asarray(inputs["wq"]),
        np.asarray(inputs["wk"]), np.asarray(inputs["wv"]),
        np.asarray(inputs["wo"]), np.asarray(inputs["freqs_cos"]),
        np.asarray(inputs["freqs_sin"]))

    res = run_bass_kernel_spmd(nc, in_maps, core_ids=list(range(N_CORES)),
                               trace=trace)
    out = np.zeros((BSZ, SEQ, DIM), np.float32)
    for c in range(N_CORES):
        out[c // 4] += res.results[c]["out"].astype(np.float32)
    return out, res


def kernel(**inputs) -> np.ndarray:
    out, _ = _run(inputs, trace=False)
    return out


# revision 30
# speedup vs baseline: 1.0072x; 1.0048x over previous
"""Distributed GQA attention prefill kernel for 8 TRN2 NeuronCores.

Problem: llama-style attention, BSZ=2, SEQ=2048, DIM=4096, 32 Q heads,
8 KV heads, head_dim=128, causal prefill (start_pos=0, caches zero).

Sharding: data-parallel over batch (2) x tensor-parallel over heads (4).
Core c = (b, g) with b = c // 4, g = c % 4 handles batch b, Q heads
8g..8g+7, KV heads 2g..2g+1, and wo rows 1024g..1024(g+1). Each core
emits a partial [2048, 4096] output; the host sums the 4 TP partials
per batch. No collectives.

On-chip layout trick: everything is computed in "transposed" layouts so
no activation transpose is ever needed:
  QT[d, t] = wq.T @ x.T       (lhsT = wq natural, rhs = xT from host)
  KT[d, t] = wk.T @ x.T
  V[t, d]  = x @ wv           (lhsT = xT chunk, rhs = wv natural)
  scoresT[kv, q] = K @ QT     (lhsT = KT tile, rhs = QT tile)
  attn[q, d+1]   = P @ [V|1]  (lhsT = expT tile, rhs = V with ones col
                               -> last column accumulates the softmax
                               denominator for free)
RoPE is applied in rotate-half form: the head_dim of wq/wk is permuted
on the host (even dims first, odd dims second) which leaves all dot
products unchanged; cos/sin arrive transposed [64, t].

Schedule: stage A does Q/K/V projection in one x-pass (QT spilled to
DRAM); stage B (attention) then runs with stage C (output projection)
interleaved at head granularity — C's matmuls fill the PE while ScalarE
computes the next head's exp row.
"""

import sys

for p in ("/opt/pypackages", "/opt/trn_rl_repo"):
    if p not in sys.path:
        sys.path.insert(0, p)

import numpy as np
import ml_dtypes

BSZ, SEQ, DIM = 2, 2048, 4096
N_HEADS, N_KV, HD = 32, 8, 128
H_L, KV_L = 8, 2          # per-core local Q heads / KV heads
HL = H_L * HD             # 1024 local head dims
N_CORES = 8
WIN = 256                 # stage-A token window
NW = SEQ // WIN
NK = DIM // 128
NQT = SEQ // 512          # attention q-tiles
NEG = -1e9

_cache = {}


def _build():
    import concourse.mybir as mybir
    import concourse.tile as tile
    from concourse import bacc
    from concourse.masks import make_identity
    from contextlib import ExitStack

    f32 = mybir.dt.float32
    bf16 = mybir.dt.bfloat16
    Exp = mybir.ActivationFunctionType.Exp

    nc = bacc.Bacc()
    xT = nc.declare_dram_parameter("xT", [DIM, SEQ], bf16, isOutput=False)
    wq = nc.declare_dram_parameter("wq", [DIM, HL], bf16, isOutput=False)
    wk = nc.declare_dram_parameter("wk", [DIM, KV_L * HD], bf16, isOutput=False)
    wv = nc.declare_dram_parameter("wv", [DIM, KV_L * HD], bf16, isOutput=False)
    wo = nc.declare_dram_parameter("wo", [HL, DIM], bf16, isOutput=False)
    cosT = nc.declare_dram_parameter("cosT", [64, SEQ], f32, isOutput=False)
    sinT = nc.declare_dram_parameter("sinT", [64, SEQ], f32, isOutput=False)
    maskb = nc.declare_dram_parameter("maskb", [128, 4, 512], f32, isOutput=False)
    out = nc.declare_dram_parameter("out", [SEQ, DIM], f32, isOutput=True)

    qt_dram = nc.dram_tensor("qt_spill", [H_L, HD, SEQ], bf16)

    def dma_split(dst, src, n):
        """Issue n parallel DMAs over the ko axis (dim 1 of dst)."""
        ko = dst.shape[1]
        step = ko // n
        for i in range(n):
            nc.sync.dma_start(
                out=dst[:, i * step:(i + 1) * step],
                in_=src[:, i * step:(i + 1) * step])

    with tile.TileContext(nc) as tc, ExitStack() as res:
        ps_big = res.enter_context(tc.tile_pool(name="ps_big", bufs=4, space="PSUM"))
        ps_att = res.enter_context(tc.tile_pool(name="ps_att", bufs=4, space="PSUM"))
        resid = res.enter_context(tc.tile_pool(name="resid", bufs=1))
        qt_pool = res.enter_context(tc.tile_pool(name="qt", bufs=2))
        qts = {}

        # per-window K/V tiles (window = 512 tokens for attention indexing)
        kt_w = [resid.tile([128, KV_L, 512], bf16, tag=f"kt{w}", name=f"kt{w}")
                for w in range(NQT)]
        v_w = [resid.tile([128, 4, KV_L, 130], bf16, tag=f"v{w}",
                          name=f"v{w}") for w in range(NQT)]
        ident = resid.tile([128, 128], bf16, tag="ident")
        cos_sb = resid.tile([64, SEQ], f32, tag="cos")
        sin_sb = resid.tile([64, SEQ], f32, tag="sin")
        mask_sb = resid.tile([128, 4, 512], f32, tag="mask")

        def rope(ps, dst, t0, tw):
            """dst[0:64]=e*c-o*s ; dst[64:128]=e*s+o*c (e=ps[0:64], o=ps[64:128])."""
            c = cos_sb[:, t0:t0 + tw]
            s = sin_sb[:, t0:t0 + tw]
            t1 = rope_pool.tile([64, WIN], f32, tag="r1", name="r1")[:, :tw]
            t2 = rope_pool.tile([64, WIN], f32, tag="r2", name="r2")[:, :tw]
            nc.vector.tensor_mul(t1, ps[0:64, :tw], c)
            nc.vector.tensor_mul(t2, ps[64:128, :tw], s)
            nc.vector.tensor_sub(dst[0:64, :tw], t1, t2)
            t3 = rope_pool.tile([64, WIN], f32, tag="r1", name="r3")[:, :tw]
            t4 = rope_pool.tile([64, WIN], f32, tag="r2", name="r4")[:, :tw]
            nc.vector.tensor_mul(t3, ps[0:64, :tw], s)
            nc.vector.tensor_mul(t4, ps[64:128, :tw], c)
            nc.vector.tensor_add(dst[64:128, :tw], t3, t4)

        # ---- stage A: Q/K/V projection + RoPE in one x-pass ---------------
        with ExitStack() as sa:
            wq_sb = sa.enter_context(tc.tile_pool(name="wq", bufs=1)).tile(
                [128, NK, HL], bf16, tag="wq")
            wk_sb = sa.enter_context(tc.tile_pool(name="wk", bufs=1)).tile(
                [128, NK, KV_L * HD], bf16, tag="wk")
            wv_sb = sa.enter_context(tc.tile_pool(name="wv", bufs=1)).tile(
                [128, NK, KV_L * HD], bf16, tag="wv")
            xt_pool = sa.enter_context(tc.tile_pool(name="xt", bufs=2))
            rope_pool = sa.enter_context(tc.tile_pool(name="rope", bufs=2))
            qsp_pool = sa.enter_context(tc.tile_pool(name="qsp", bufs=2))

            # critical-path loads first: window-0 xT + wk get the DMA queues
            # to themselves so the first matmul starts in ~2us; wq's 8MB is
            # emitted after window 0's K/V work (Q is last in the window).
            # tiny first chunks so the very first matmul's operands land fast
            xt0 = xt_pool.tile([128, NK, WIN], bf16, tag="xt", name="xt0")
            xt0_src = xT[:, 0:WIN].rearrange("(ko p) t -> p ko t", p=128)
            wk_src = wk.rearrange("(ko p) d -> p ko d", p=128)
            nc.sync.dma_start(out=xt0[:, 0:2], in_=xt0_src[:, 0:2])
            nc.sync.dma_start(out=wk_sb[:, 0:2], in_=wk_src[:, 0:2])
            dma_split(xt0[:, 2:], xt0_src[:, 2:], 6)
            dma_split(wk_sb[:, 2:], wk_src[:, 2:], 6)
            dma_split(wv_sb, wv.rearrange("(ko p) d -> p ko d", p=128), 8)
            nc.sync.dma_start(out=cos_sb, in_=cosT[:, :])
            nc.sync.dma_start(out=sin_sb, in_=sinT[:, :])
            make_identity(nc, ident)
            for w_ in range(NQT):
                nc.vector.memset(v_w[w_][:, :, :, 128:129], 1.0)

            for w in range(NW):
                t0 = w * WIN
                wa, wo512 = t0 // 512, (t0 % 512)
                if w == 0:
                    xt = xt0
                else:
                    xt = xt_pool.tile([128, NK, WIN], bf16, tag="xt", name="xt")
                    dma_split(xt, xT[:, t0:t0 + WIN].rearrange(
                        "(ko p) t -> p ko t", p=128), 4)
                for kh in range(KV_L):
                    ps = ps_big.tile([128, 512], f32, tag="big", name="psk")
                    for k in range(NK):
                        nc.tensor.matmul(
                            ps[:, :WIN], wk_sb[:, k, kh * HD:(kh + 1) * HD],
                            xt[:, k], start=(k == 0), stop=(k == NK - 1))
                    rope(ps, kt_w[wa][:, kh, wo512:wo512 + WIN], t0, WIN)
                for tc_ in range(WIN // 128):
                    ps = ps_big.tile([128, 512], f32, tag="big", name="psv")
                    for k in range(NK):
                        nc.tensor.matmul(
                            ps[:, :KV_L * HD], xt[:, k, tc_ * 128:(tc_ + 1) * 128],
                            wv_sb[:, k], start=(k == 0), stop=(k == NK - 1))
                    for kh in range(KV_L):
                        nc.scalar.copy(
                            v_w[wa][:, wo512 // 128 + tc_, kh, 0:128],
                            ps[:, kh * HD:(kh + 1) * HD])
                def q_unit(xt_, t0_, h):
                    ps = ps_big.tile([128, 512], f32, tag="big", name="psq")
                    for k in range(NK):
                        nc.tensor.matmul(
                            ps[:, :WIN], wq_sb[:, k, h * HD:(h + 1) * HD],
                            xt_[:, k], start=(k == 0), stop=(k == NK - 1))
                    qs = qsp_pool.tile([128, WIN], bf16, tag="qs", name="qs")
                    rope(ps, qs, t0_, WIN)
                    nc.sync.dma_start(out=qt_dram[h, :, t0_:t0_ + WIN], in_=qs)

                if w == 0:
                    # wq in column halves: heads 4-7 of window 0 are deferred
                    # into window 1, halving the wq bytes window 0 must wait on
                    wq_src = wq.rearrange("(ko p) d -> p ko d", p=128)
                    for i in range(4):
                        nc.sync.dma_start(
                            out=wq_sb[:, i * 8:(i + 1) * 8, 0:512],
                            in_=wq_src[:, i * 8:(i + 1) * 8, 0:512])
                    for i in range(4):
                        nc.sync.dma_start(
                            out=wq_sb[:, i * 8:(i + 1) * 8, 512:1024],
                            in_=wq_src[:, i * 8:(i + 1) * 8, 512:1024])
                    nc.sync.dma_start(out=mask_sb, in_=maskb[:, :, :])
                    for h in range(4):
                        q_unit(xt, t0, h)
                else:
                    if w == 1:
                        for h in range(4, H_L):
                            q_unit(xt0, 0, h)
                    for h in range(H_L):
                        q_unit(xt, t0, h)
                if t0 + WIN == 1024:
                    # first attention q-tile (q1) fully spilled -> prefetch it
                    qts[1] = qt_pool.tile([128, H_L, 512], bf16, tag="qt",
                                          name="qt0")
                    dma_split(qts[1], qt_dram[:, :, 512:1024]
                              .rearrange("h p q -> p h q"), 2)

        # ---- stage B (attention) with stage C (out-proj) interleaved ------
        with ExitStack() as bc:
            exp_pool = bc.enter_context(tc.tile_pool(name="exp", bufs=8))
            asb_pool = bc.enter_context(tc.tile_pool(name="asb", bufs=8))
            rec_pool = bc.enter_context(tc.tile_pool(name="rec", bufs=8))
            at_sb = bc.enter_context(tc.tile_pool(name="at", bufs=1)).tile(
                [128, H_L, SEQ], bf16, tag="at")
            wo_pool = bc.enter_context(tc.tile_pool(name="wo", bufs=2))
            out_pool = bc.enter_context(tc.tile_pool(name="outp", bufs=4))

            pending = []

            def flush_pending():
                while pending:
                    pending.pop(0)()

            wo_cur = [None]

            def make_strip(qs_):
                """Emission closures for out-proj of token strip qs_ (4 ti)."""
                cls = []
                for di in range(DIM // 512):
                    def load_wo(di=di):
                        wot = wo_pool.tile([128, H_L, 512], bf16, tag="wo",
                                           name="wot")
                        dma_split(wot, wo[:, di * 512:(di + 1) * 512].rearrange(
                            "(ho p) d -> p ho d", p=128), 2)
                        wo_cur[0] = wot
                    cls.append(load_wo)
                    for tj in range(4):
                        def pair(di=di, ti=qs_ * 4 + tj):
                            wot = wo_cur[0]
                            # ps_att: don't contend with exp-pending score
                            # tiles in ps_big while ACT drains them
                            ps = ps_att.tile([128, 512], f32, tag="att",
                                             name="pso")
                            for ho in range(H_L):
                                nc.tensor.matmul(
                                    ps, at_sb[:, ho, ti * 128:(ti + 1) * 128],
                                    wot[:, ho], start=(ho == 0),
                                    stop=(ho == H_L - 1))
                            osb = out_pool.tile([128, 512], f32, tag="osb",
                                                name="osb")
                            nc.vector.tensor_copy(osb, ps)
                            nc.sync.dma_start(
                                out=out[ti * 128:(ti + 1) * 128,
                                        di * 512:(di + 1) * 512],
                                in_=osb)
                        cls.append(pair)
                return cls

            cqueue = []

            # q1 first so its finished strip feeds PE fillers during B(q0)
            order = [1, 0, 2, 3]

            for idx, qi in enumerate(order):
                q0 = qi * 512
                qt = qts.pop(qi)
                if idx + 1 < len(order):
                    nq = order[idx + 1]
                    qts[nq] = qt_pool.tile([128, H_L, 512], bf16,
                                           tag="qt", name="qtn")
                    dma_split(qts[nq], qt_dram[:, :, nq * 512:nq * 512 + 512]
                              .rearrange("h p q -> p h q"), 2)
                if idx >= 1:
                    cqueue.extend(make_strip(order[idx - 1]))
                for h in range(H_L):
                    kh = h // 4
                    nkv = 4 * (qi + 1)
                    pes = []
                    for kvt in range(nkv):
                        # columns q < r*128 of a diagonal tile are fully masked
                        r = kvt - 4 * qi
                        c0 = max(r, 0) * 128
                        ps = ps_big.tile([128, 512], f32, tag="big", name="pss")
                        nc.tensor.matmul(
                            ps[:, c0:], kt_w[kvt // 4][:, kh,
                                               (kvt % 4) * 128:(kvt % 4 + 1) * 128],
                            qt[:, h, c0:], start=True, stop=True)
                        if r >= 0:
                            nc.vector.tensor_add(ps[:, c0:], ps[:, c0:],
                                                 mask_sb[:, r, c0:])
                        pe = exp_pool.tile([128, 512], bf16, tag="exp", name="pe")
                        nc.scalar.activation(pe[:, c0:], ps[:, c0:], Exp)
                        pes.append(pe)
                        if kvt == 3:
                            flush_pending()
                    # PE filler while ScalarE computes this head's exps
                    for _ in range(5):
                        if cqueue:
                            cqueue.pop(0)()
                    flush_pending()
                    aps = [ps_att.tile([128, 129], f32, tag="att", name=f"att{_qc}")
                           for _qc in range(4)]
                    for kvt in range(nkv):
                        for qc in range(4):
                            if qc < kvt - 4 * qi:
                                continue  # q-chunk entirely masked for this kv
                            nc.tensor.matmul(
                                aps[qc], pes[kvt][:, qc * 128:(qc + 1) * 128],
                                v_w[kvt // 4][:, kvt % 4, kh, 0:129],
                                start=(kvt == 0), stop=(kvt == 4 * qi + qc))
                    asbs = []
                    for qc in range(4):
                        rec = rec_pool.tile([128, 1], f32, tag="rec", name="rec")
                        nc.vector.reciprocal(rec, aps[qc][:, 128:129])
                        asb = asb_pool.tile([128, 128], bf16, tag="asb", name="asb")
                        nc.vector.tensor_scalar_mul(asb, aps[qc][:, 0:128], rec)
                        asbs.append(asb)

                    def defer(h=h, q0=q0, asbs=asbs):
                        for qc in range(4):
                            pst = ps_att.tile([128, 128], bf16, tag="att",
                                              name="pst")
                            nc.tensor.transpose(pst, asbs[qc], ident)
                            nc.vector.tensor_copy(
                                at_sb[:, h, q0 + qc * 128:q0 + (qc + 1) * 128],
                                pst)
                    pending.append(defer)
                flush_pending()
            cqueue.extend(make_strip(order[-1]))
            for c in cqueue:
                c()

    nc.finalize()
    return nc


def _prep_inputs(x, wq, wk, wv, wo, freqs_cos, freqs_sin):
    """Host-side shard prep. Returns in_maps for cores 0..7."""
    bf = ml_dtypes.bfloat16
    perm = np.concatenate([np.arange(0, HD, 2), np.arange(1, HD, 2)])  # rotate-half

    wq_p = (wq.astype(np.float32) / np.sqrt(HD)).reshape(DIM, N_HEADS, HD)[:, :, perm]
    wk_p = wk.astype(np.float32).reshape(DIM, N_KV, HD)[:, :, perm]

    cosT = np.ascontiguousarray(freqs_cos.astype(np.float32).T)  # [64, SEQ]
    sinT = np.ascontiguousarray(freqs_sin.astype(np.float32).T)

    # causal band mask tiles: maskb[kvl, r, ql] = 0 if r*128+kvl <= ql else NEG
    kvl = np.arange(128)[:, None, None]
    r = np.arange(4)[None, :, None]
    ql = np.arange(512)[None, None, :]
    maskb = np.where(r * 128 + kvl <= ql, 0.0, NEG).astype(np.float32)

    xTs = [np.ascontiguousarray(x[b].astype(np.float32).T).astype(bf)
           for b in range(BSZ)]

    in_maps = []
    for c in range(N_CORES):
        b, g = c // 4, c % 4
        in_maps.append({
            "xT": xTs[b],
            "wq": np.ascontiguousarray(
                wq_p[:, g * H_L:(g + 1) * H_L].reshape(DIM, HL)).astype(bf),
            "wk": np.ascontiguousarray(
                wk_p[:, g * KV_L:(g + 1) * KV_L].reshape(DIM, KV_L * HD)).astype(bf),
            "wv": np.ascontiguousarray(
                wv[:, g * KV_L * HD:(g + 1) * KV_L * HD]).astype(bf),
            "wo": np.ascontiguousarray(
                wo[g * HL:(g + 1) * HL]).astype(bf),
            "cosT": cosT,
            "sinT": sinT,
            "maskb": maskb,
        })
    return in_maps


def _run(inputs, trace=False):
    from concourse.bass_utils import run_bass_kernel_spmd

    if "nc" not in _cache:
        _cache["nc"] = _build()
    nc = _cache["nc"]

    in_maps = _prep_inputs(
        np.asarray(inputs["x"]), np.asarray(inputs["wq"]),
        np.asarray(inputs["wk"]), np.asarray(inputs["wv"]),
        np.asarray(inputs["wo"]), np.asarray(inputs["freqs_cos"]),
        np.asarray(inputs["freqs_sin"]))

    res = run_bass_kernel_spmd(nc, in_maps, core_ids=list(range(N_CORES)),
                               trace=trace)
    out = np.zeros((BSZ, SEQ, DIM), np.float32)
    for c in range(N_CORES):
        out[c // 4] += res.results[c]["out"].astype(np.float32)
    return out, res


def kernel(**inputs) -> np.ndarray:
    out, _ = _run(inputs, trace=False)
    return out
